# revision 2
# baseline (speedup 1.0000x reference)
"""Trainium2 Bass kernel for nn_PolicyEstimator: 2-layer LSTM (H=2048) run
autoregressively for 16 steps, batch=1, softmax output fed back as input.

Strategy (8 NeuronCores, tensor-parallel per the gate-dim sharding hint):
- Each core owns H-slice c (256 of 2048 hidden units): the i/f/g/o gate rows
  for that slice of W_ih/W_hh (layers 0,1), plus the matching 256 rows of
  W_out. All weights live SBUF-resident in fp16 (~18MB/core), loaded once.
- Matvecs run x-stationary on the tensor engine with 4-way column tiling:
  the activation vector is the stationary operand ([128,1] per K-chunk) and
  the weight streams as the moving operand; col-tile j computes gate j's
  strip, so i/f/g/o land on PSUM partitions 0/32/64/96.
- Biases are folded into the matmul as a 17th K-chunk (lhsT column = e0,
  rhs chunk row 0 = bias).
- Per step: AllGather(h1) -> layer2, AllGather(h2) -> W_out, AllGather(logit
  slices) -> replicated softmax on every core (fp32), which becomes the next
  step's input. Cell state c stays core-local; never communicated.
"""
import sys

sys.path.insert(0, "/opt/trn_rl_repo")
import numpy as np

S = 2048
H = 2048
L = 2
A = 16
N_CORES = 8
HS = H // N_CORES        # 256 hidden units per core
NG = 4 * HS              # 1024 gate rows per core
KC = H // 128            # 16 contraction chunks
NSTRIP = NG // 4         # 256 gate values per col-tile strip
OSTRIP = HS // 4         # 64 logits per col-tile strip

_CACHE = {}


def _build():
    from concourse.bacc import Bacc
    import concourse.mybir as mybir
    from concourse.tile import TileContext

    f16 = mybir.dt.float16
    f32 = mybir.dt.float32
    AF = mybir.ActivationFunctionType

    nc = Bacc(num_devices=N_CORES)

    wih0_in = nc.dram_tensor("wih0", [KC + 1, 128, NG], f16, kind="ExternalInput")
    whh0_in = nc.dram_tensor("whh0", [KC, 128, NG], f16, kind="ExternalInput")
    wih1_in = nc.dram_tensor("wih1", [KC + 1, 128, NG], f16, kind="ExternalInput")
    whh1_in = nc.dram_tensor("whh1", [KC, 128, NG], f16, kind="ExternalInput")
    wout_in = nc.dram_tensor("wout", [KC + 1, 128, HS], f16, kind="ExternalInput")
    x_in = nc.dram_tensor("x0", [128, KC + 1], f16, kind="ExternalInput")
    preds = nc.dram_tensor("preds", [A, S], f32, kind="ExternalOutput")

    cc_h1_in = nc.dram_tensor("cc_h1_in", [HS], f16, kind="Internal")
    cc_h1_out = nc.dram_tensor("cc_h1_out", [H], f16, kind="Internal", addr_space="Shared")
    cc_h2_in = nc.dram_tensor("cc_h2_in", [HS], f16, kind="Internal")
    cc_h2_out = nc.dram_tensor("cc_h2_out", [H], f16, kind="Internal", addr_space="Shared")
    cc_lg_in = nc.dram_tensor("cc_lg_in", [HS], f32, kind="Internal")
    cc_lg_out = nc.dram_tensor("cc_lg_out", [S], f32, kind="Internal", addr_space="Shared")
    RG = [list(range(N_CORES))]

    with TileContext(nc) as tc:
        with tc.tile_pool(name="w", bufs=1) as wp, \
             tc.tile_pool(name="act", bufs=1) as ap, \
             tc.tile_pool(name="pg1", bufs=2, space="PSUM") as pg1, \
             tc.tile_pool(name="pg2", bufs=2, space="PSUM") as pg2, \
             tc.tile_pool(name="po", bufs=1, space="PSUM") as po, \
             tc.tile_pool(name="pb", bufs=1, space="PSUM") as pb:

            # ---- resident weights
            Wih0 = wp.tile([128, (KC + 1) * NG], f16)
            Whh0 = wp.tile([128, KC * NG], f16)
            Wih1 = wp.tile([128, (KC + 1) * NG], f16)
            Whh1 = wp.tile([128, KC * NG], f16)
            Wout = wp.tile([128, (KC + 1) * HS], f16)
            for tile, src, nch, ng in (
                (Wih0, wih0_in, KC + 1, NG),
                (Whh0, whh0_in, KC, NG),
                (Wih1, wih1_in, KC + 1, NG),
                (Whh1, whh1_in, KC, NG),
                (Wout, wout_in, KC + 1, HS),
            ):
                nc.sync.dma_start(
                    tile.rearrange("p (c n) -> p c n", c=nch),
                    src.rearrange("c p n -> p c n"),
                )

            # ---- activation vectors (lhsT layout [128, 17]; col16 = e0 for bias)
            xt = wp.tile([128, KC + 1], f16)
            nc.sync.dma_start(xt[:], x_in[:])
            h1t = wp.tile([128, KC + 1], f16)
            h2t = wp.tile([128, KC + 1], f16)
            for ht in (h1t, h2t):
                nc.vector.memset(ht[:, KC : KC + 1], 0.0)
                nc.vector.memset(ht[0:1, KC : KC + 1], 1.0)

            # ---- elementwise workspace (all at partition 0; [1, HS] tiles)
            ew = []
            for l in range(L):
                tiles = {}
                for name in ("si", "sf", "tg", "so", "pa", "qa", "tcx"):
                    tiles[name] = ap.tile([1, HS], f32, tag=f"ew{l}_{name}", name=f"ew{l}_{name}")
                tiles["cst"] = ap.tile([1, HS], f32, tag=f"cst{l}", name=f"cst{l}")
                nc.vector.memset(tiles["cst"][:], 0.0)
                tiles["hs16"] = ap.tile([1, HS], f16, tag=f"hs{l}", name=f"hs{l}")
                ew.append(tiles)

            LS = ap.tile([1, HS], f32)
            LG = ap.tile([128, KC], f32)
            E = ap.tile([128, KC], f32)
            SM = ap.tile([128, 1], f32)
            SB = ap.tile([128, 1], f32)
            XF = ap.tile([128, KC], f32)
            ones32 = ap.tile([128, 32], f32)
            nc.vector.memset(ones32[:], 1.0)

            def matvec(ps, W, nch, lhsT, started, ng, nstrip, stop=False):
                for c in range(nch):
                    for j in range(4):
                        nc.tensor.matmul(
                            ps[32 * j : 32 * j + 1, :],
                            lhsT=lhsT[:, c : c + 1],
                            rhs=W[:, c * ng + j * nstrip : c * ng + (j + 1) * nstrip],
                            start=not started[j],
                            stop=stop and (c == nch - 1),
                            tile_position=(0, 32 * j),
                        )
                        started[j] = True

            def lstm_ew(ps, l):
                t = ew[l]
                nc.scalar.activation(t["si"][:], ps[0:1, :], AF.Sigmoid)
                nc.scalar.activation(t["sf"][:], ps[32:33, :], AF.Sigmoid)
                nc.scalar.activation(t["tg"][:], ps[64:65, :], AF.Tanh)
                nc.scalar.activation(t["so"][:], ps[96:97, :], AF.Sigmoid)
                nc.vector.tensor_mul(t["pa"][:], t["si"][:], t["tg"][:])
                nc.vector.tensor_mul(t["qa"][:], t["sf"][:], t["cst"][:])
                nc.vector.tensor_add(t["cst"][:], t["pa"][:], t["qa"][:])
                nc.scalar.activation(t["tcx"][:], t["cst"][:], AF.Tanh)
                nc.vector.tensor_mul(t["hs16"][:], t["so"][:], t["tcx"][:])
                return t["hs16"]

            row1 = lambda d: d.rearrange("(o f) -> o f", o=1)
            grid = lambda d: d.rearrange("(p f) -> p f", p=128)

            ps1 = pg1.tile([128, NSTRIP], f32, tag="g1", name="ps1")
            st1 = [False] * 4
            for t in range(A):
                # layer 1: gates = W_ih0 @ x (+ W_hh0 @ h1_prev, prefetched last step)
                matvec(ps1, Wih0, KC + 1, xt, st1, NG, NSTRIP, stop=True)
                hs1 = lstm_ew(ps1, 0)
                nc.sync.dma_start(row1(cc_h1_in), hs1[:])
                nc.gpsimd.collective_compute(
                    "AllGather", mybir.AluOpType.bypass, replica_groups=RG,
                    ins=[cc_h1_in[:]], outs=[cc_h1_out[:]],
                )

                # layer 2 recurrent part can run during the AllGather
                ps2 = pg2.tile([128, NSTRIP], f32, tag="g2", name="ps2")
                st2 = [False] * 4
                if t > 0:
                    matvec(ps2, Whh1, KC, h2t, st2, NG, NSTRIP)

                nc.sync.dma_start(h1t[:, 0:KC], grid(cc_h1_out))
                matvec(ps2, Wih1, KC + 1, h1t, st2, NG, NSTRIP, stop=True)
                hs2 = lstm_ew(ps2, 1)
                nc.sync.dma_start(row1(cc_h2_in), hs2[:])
                nc.gpsimd.collective_compute(
                    "AllGather", mybir.AluOpType.bypass, replica_groups=RG,
                    ins=[cc_h2_in[:]], outs=[cc_h2_out[:]],
                )

                # prefetch next step's layer-1 recurrent matvec during the AllGather
                if t < A - 1:
                    ps1n = pg1.tile([128, NSTRIP], f32, tag="g1", name="ps1n")
                    st1n = [False] * 4
                    matvec(ps1n, Whh0, KC, h1t, st1n, NG, NSTRIP)

                nc.sync.dma_start(h2t[:, 0:KC], grid(cc_h2_out))
                ps3 = po.tile([128, OSTRIP], f32, tag="o", name="ps3")
                st3 = [False] * 4
                matvec(ps3, Wout, KC + 1, h2t, st3, HS, OSTRIP, stop=True)
                for j in range(4):
                    nc.scalar.activation(
                        LS[0:1, j * OSTRIP : (j + 1) * OSTRIP],
                        ps3[32 * j : 32 * j + 1, :], AF.Copy,
                    )
                nc.sync.dma_start(row1(cc_lg_in), LS[:])
                nc.gpsimd.collective_compute(
                    "AllGather", mybir.AluOpType.bypass, replica_groups=RG,
                    ins=[cc_lg_in[:]], outs=[cc_lg_out[:]],
                )
                nc.sync.dma_start(LG[:], grid(cc_lg_out))

                # replicated softmax (logits are small; exp without max-shift)
                nc.scalar.activation(E[:], LG[:], AF.Exp, accum_out=SM[:])
                psb = pb.tile([128, 1], f32, tag="b", name="psb")
                for j in range(4):
                    nc.tensor.matmul(
                        psb[32 * j : 32 * (j + 1), :], lhsT=ones32[:], rhs=SM[:],
                        start=True, stop=True, tile_position=(0, 32 * j),
                    )
                nc.vector.reciprocal(SB[:], psb[:])
                nc.scalar.activation(xt[:, 0:KC], E[:], AF.Copy, scale=SB[:])
                nc.vector.tensor_scalar_mul(XF[:], E[:], SB[:])
                nc.sync.dma_start(grid(preds[t]), XF[:])

                if t < A - 1:
                    ps1 = ps1n
                    st1 = st1n
    nc.finalize()
    return nc


def _pack_mat(Wsh, bias=None):
    """Wsh [rows, H] -> [(KC + has_bias), 128, rows] fp16, K-permuted so that
    chunk j row p holds contraction index k = 16p + j (matches the [128,16]
    row-major activation layout)."""
    rows = Wsh.shape[0]
    WT = np.ascontiguousarray(Wsh.T)  # [H, rows]
    out = WT.reshape(128, KC, rows).transpose(1, 0, 2)  # [KC, 128, rows]
    if bias is not None:
        bc = np.zeros((1, 128, rows), np.float32)
        bc[0, 0, :] = bias
        out = np.concatenate([out, bc], axis=0)
    return np.ascontiguousarray(out.astype(np.float16))


def _prep_inputs(x, W_ih, W_hh, b_ih, b_hh, W_out, b_out):
    in_maps = []
    e0 = np.zeros((128, 1), np.float32)
    e0[0, 0] = 1.0
    xdev = np.concatenate([x.reshape(128, KC), e0], axis=1).astype(np.float16)
    for c in range(N_CORES):
        sl = slice(c * HS, (c + 1) * HS)
        m = {}
        for l, (ih_name, hh_name) in enumerate((("wih0", "whh0"), ("wih1", "whh1"))):
            Wi = W_ih[l].reshape(4, H, H)[:, sl, :].reshape(NG, H)
            Wh = W_hh[l].reshape(4, H, H)[:, sl, :].reshape(NG, H)
            b = (b_ih[l] + b_hh[l]).reshape(4, H)[:, sl].reshape(NG)
            m[ih_name] = _pack_mat(Wi, b)
            m[hh_name] = _pack_mat(Wh)
        m["wout"] = _pack_mat(W_out[sl, :], b_out[sl])
        m["x0"] = xdev
        in_maps.append(m)
    return in_maps


def run(trace=False, **inputs):
    from concourse.bass_utils import run_bass_kernel_spmd

    if "nc" not in _CACHE:
        _CACHE["nc"] = _build()
    nc = _CACHE["nc"]
    inputs = {k: np.asarray(v, dtype=np.float32) for k, v in inputs.items()}
    in_maps = _prep_inputs(**inputs)
    res = run_bass_kernel_spmd(
        nc, in_maps, core_ids=list(range(N_CORES)), trace=trace,
    )
    return res


def kernel(**inputs):
    res = run(trace=False, **inputs)
    return np.asarray(res.results[0]["preds"], dtype=np.float32)


# revision 3
# speedup vs baseline: 36.0282x; 36.0282x over previous
"""Trainium2 Bass kernel for nn_PolicyEstimator: 2-layer LSTM (H=2048) run
autoregressively for 16 steps, batch=1, softmax output fed back as input.

Strategy (8 NeuronCores, tensor-parallel per the gate-dim sharding hint):
- Each core owns H-slice c (256 of 2048 hidden units): the i/f/g/o gate rows
  for that slice of W_ih/W_hh (layers 0,1), plus the matching 256 rows of
  W_out. All weights live SBUF-resident in fp16 (~18MB/core), loaded once.
- Matvecs run x-stationary on the tensor engine with 4-way column tiling:
  the activation vector is the stationary operand ([128,1] per K-chunk) and
  the weight streams as the moving operand; col-tile j computes gate j's
  strip, so i/f/g/o land on PSUM partitions 0/32/64/96.
- Biases are folded into the matmul as a 17th K-chunk (lhsT column = e0,
  rhs chunk row 0 = bias).
- Per step: AllGather(h1) -> layer2, AllGather(h2) -> W_out, AllGather(logit
  slices) -> replicated softmax on every core (fp32), which becomes the next
  step's input. Cell state c stays core-local; never communicated.
"""
import sys

sys.path.insert(0, "/opt/trn_rl_repo")
import numpy as np

S = 2048
H = 2048
L = 2
A = 16
N_CORES = 8
HS = H // N_CORES        # 256 hidden units per core
NG = 4 * HS              # 1024 gate rows per core
KC = H // 128            # 16 contraction chunks
NSTRIP = NG // 4         # 256 gate values per col-tile strip
OSTRIP = HS // 4         # 64 logits per col-tile strip

_CACHE = {}


def _build(repeats=1):
    from concourse.bacc import Bacc
    import concourse.mybir as mybir
    from concourse.tile import TileContext

    f16 = mybir.dt.float16
    f32 = mybir.dt.float32
    AF = mybir.ActivationFunctionType

    nc = Bacc(num_devices=N_CORES)

    wih0_in = nc.dram_tensor("wih0", [KC + 1, 128, NG], f16, kind="ExternalInput")
    whh0_in = nc.dram_tensor("whh0", [KC, 128, NG], f16, kind="ExternalInput")
    wih1_in = nc.dram_tensor("wih1", [KC + 1, 128, NG], f16, kind="ExternalInput")
    whh1_in = nc.dram_tensor("whh1", [KC, 128, NG], f16, kind="ExternalInput")
    wout_in = nc.dram_tensor("wout", [KC + 1, 128, HS], f16, kind="ExternalInput")
    x_in = nc.dram_tensor("x0", [128, KC + 1], f16, kind="ExternalInput")
    preds = nc.dram_tensor("preds", [A, S], f32, kind="ExternalOutput")

    cc_h1_in = nc.dram_tensor("cc_h1_in", [HS], f16, kind="Internal")
    cc_h1_out = nc.dram_tensor("cc_h1_out", [H], f16, kind="Internal", addr_space="Shared")
    cc_h2_in = nc.dram_tensor("cc_h2_in", [HS], f16, kind="Internal")
    cc_h2_out = nc.dram_tensor("cc_h2_out", [H], f16, kind="Internal", addr_space="Shared")
    cc_lg_in = nc.dram_tensor("cc_lg_in", [HS], f32, kind="Internal")
    cc_lg_out = nc.dram_tensor("cc_lg_out", [S], f32, kind="Internal", addr_space="Shared")
    RG = [list(range(N_CORES))]

    with TileContext(nc) as tc:
        with tc.tile_pool(name="w", bufs=1) as wp, \
             tc.tile_pool(name="act", bufs=1) as ap, \
             tc.tile_pool(name="pg1", bufs=2, space="PSUM") as pg1, \
             tc.tile_pool(name="pg2", bufs=2, space="PSUM") as pg2, \
             tc.tile_pool(name="po", bufs=1, space="PSUM") as po, \
             tc.tile_pool(name="pb", bufs=1, space="PSUM") as pb:

            # ---- resident weights
            Wih0 = wp.tile([128, (KC + 1) * NG], f16)
            Whh0 = wp.tile([128, KC * NG], f16)
            Wih1 = wp.tile([128, (KC + 1) * NG], f16)
            Whh1 = wp.tile([128, KC * NG], f16)
            Wout = wp.tile([128, (KC + 1) * HS], f16)
            for tile, src, nch, ng in (
                (Wih0, wih0_in, KC + 1, NG),
                (Whh0, whh0_in, KC, NG),
                (Wih1, wih1_in, KC + 1, NG),
                (Whh1, whh1_in, KC, NG),
                (Wout, wout_in, KC + 1, HS),
            ):
                nc.sync.dma_start(
                    tile.rearrange("p (c n) -> p c n", c=nch),
                    src.rearrange("c p n -> p c n"),
                )

            # ---- activation vectors (lhsT layout [128, 17]; col16 = e0 for bias)
            xt = wp.tile([128, KC + 1], f16)
            nc.sync.dma_start(xt[:], x_in[:])
            h1t = wp.tile([128, KC + 1], f16)
            h2t = wp.tile([128, KC + 1], f16)
            for ht in (h1t, h2t):
                nc.vector.memset(ht[:, KC : KC + 1], 0.0)
                nc.vector.memset(ht[0:1, KC : KC + 1], 1.0)

            # ---- elementwise workspace (all at partition 0; [1, HS] tiles)
            ew = []
            for l in range(L):
                tiles = {}
                for name in ("si", "sf", "tg", "so", "pa", "qa", "tcx"):
                    tiles[name] = ap.tile([1, HS], f32, tag=f"ew{l}_{name}", name=f"ew{l}_{name}")
                tiles["cst"] = ap.tile([1, HS], f32, tag=f"cst{l}", name=f"cst{l}")
                nc.vector.memset(tiles["cst"][:], 0.0)
                tiles["hs16"] = ap.tile([1, HS], f16, tag=f"hs{l}", name=f"hs{l}")
                ew.append(tiles)

            LS = ap.tile([1, HS], f32)
            LG = ap.tile([128, KC], f32)
            E = ap.tile([128, KC], f32)
            SM = ap.tile([128, 1], f32)
            SB = ap.tile([128, 1], f32)
            XF = ap.tile([128, KC], f32)
            ones32 = ap.tile([128, 32], f32)
            nc.vector.memset(ones32[:], 1.0)

            def matvec(ps, W, nch, lhsT, started, ng, nstrip, stop=False):
                for c in range(nch):
                    for j in range(4):
                        nc.tensor.matmul(
                            ps[32 * j : 32 * j + 1, :],
                            lhsT=lhsT[:, c : c + 1],
                            rhs=W[:, c * ng + j * nstrip : c * ng + (j + 1) * nstrip],
                            start=not started[j],
                            stop=stop and (c == nch - 1),
                            tile_position=(0, 32 * j),
                        )
                        started[j] = True

            def lstm_ew(ps, l):
                t = ew[l]
                nc.scalar.activation(t["si"][:], ps[0:1, :], AF.Sigmoid)
                nc.scalar.activation(t["sf"][:], ps[32:33, :], AF.Sigmoid)
                nc.scalar.activation(t["tg"][:], ps[64:65, :], AF.Tanh)
                nc.scalar.activation(t["so"][:], ps[96:97, :], AF.Sigmoid)
                nc.vector.tensor_mul(t["pa"][:], t["si"][:], t["tg"][:])
                nc.vector.tensor_mul(t["qa"][:], t["sf"][:], t["cst"][:])
                nc.vector.tensor_add(t["cst"][:], t["pa"][:], t["qa"][:])
                nc.scalar.activation(t["tcx"][:], t["cst"][:], AF.Tanh)
                nc.vector.tensor_mul(t["hs16"][:], t["so"][:], t["tcx"][:])
                return t["hs16"]

            row1 = lambda d: d.rearrange("(o f) -> o f", o=1)
            grid = lambda d: d.rearrange("(p f) -> p f", p=128)

            ps1 = pg1.tile([128, NSTRIP], f32, tag="g1", name="ps1")
            st1 = [False] * 4
            for t_glob in range(A * repeats):
                t = t_glob % A
                # layer 1: gates = W_ih0 @ x (+ W_hh0 @ h1_prev, prefetched last step)
                matvec(ps1, Wih0, KC + 1, xt, st1, NG, NSTRIP, stop=True)
                hs1 = lstm_ew(ps1, 0)
                nc.sync.dma_start(row1(cc_h1_in), hs1[:])
                nc.gpsimd.collective_compute(
                    "AllGather", mybir.AluOpType.bypass, replica_groups=RG,
                    ins=[cc_h1_in[:]], outs=[cc_h1_out[:]],
                )

                # layer 2 recurrent part can run during the AllGather
                ps2 = pg2.tile([128, NSTRIP], f32, tag="g2", name="ps2")
                st2 = [False] * 4
                if t_glob > 0:
                    matvec(ps2, Whh1, KC, h2t, st2, NG, NSTRIP)

                nc.sync.dma_start(h1t[:, 0:KC], grid(cc_h1_out))
                matvec(ps2, Wih1, KC + 1, h1t, st2, NG, NSTRIP, stop=True)
                hs2 = lstm_ew(ps2, 1)
                nc.sync.dma_start(row1(cc_h2_in), hs2[:])
                nc.gpsimd.collective_compute(
                    "AllGather", mybir.AluOpType.bypass, replica_groups=RG,
                    ins=[cc_h2_in[:]], outs=[cc_h2_out[:]],
                )

                # prefetch next step's layer-1 recurrent matvec during the AllGather
                if t_glob < A * repeats - 1:
                    ps1n = pg1.tile([128, NSTRIP], f32, tag="g1", name="ps1n")
                    st1n = [False] * 4
                    matvec(ps1n, Whh0, KC, h1t, st1n, NG, NSTRIP)

                nc.sync.dma_start(h2t[:, 0:KC], grid(cc_h2_out))
                ps3 = po.tile([128, OSTRIP], f32, tag="o", name="ps3")
                st3 = [False] * 4
                matvec(ps3, Wout, KC + 1, h2t, st3, HS, OSTRIP, stop=True)
                for j in range(4):
                    nc.scalar.activation(
                        LS[0:1, j * OSTRIP : (j + 1) * OSTRIP],
                        ps3[32 * j : 32 * j + 1, :], AF.Copy,
                    )
                nc.sync.dma_start(row1(cc_lg_in), LS[:])
                nc.gpsimd.collective_compute(
                    "AllGather", mybir.AluOpType.bypass, replica_groups=RG,
                    ins=[cc_lg_in[:]], outs=[cc_lg_out[:]],
                )
                nc.sync.dma_start(LG[:], grid(cc_lg_out))

                # replicated softmax (logits are small; exp without max-shift)
                nc.scalar.activation(E[:], LG[:], AF.Exp, accum_out=SM[:])
                psb = pb.tile([128, 1], f32, tag="b", name="psb")
                for j in range(4):
                    nc.tensor.matmul(
                        psb[32 * j : 32 * (j + 1), :], lhsT=ones32[:], rhs=SM[:],
                        start=True, stop=True, tile_position=(0, 32 * j),
                    )
                nc.vector.reciprocal(SB[:], psb[:])
                nc.scalar.activation(xt[:, 0:KC], E[:], AF.Copy, scale=SB[:])
                nc.vector.tensor_scalar_mul(XF[:], E[:], SB[:])
                nc.sync.dma_start(grid(preds[t]), XF[:])

                if t_glob < A * repeats - 1:
                    ps1 = ps1n
                    st1 = st1n
    nc.finalize()
    return nc


def _pack_mat(Wsh, bias=None):
    """Wsh [rows, H] -> [(KC + has_bias), 128, rows] fp16, K-permuted so that
    chunk j row p holds contraction index k = 16p + j (matches the [128,16]
    row-major activation layout)."""
    rows = Wsh.shape[0]
    WT = np.ascontiguousarray(Wsh.T)  # [H, rows]
    out = WT.reshape(128, KC, rows).transpose(1, 0, 2)  # [KC, 128, rows]
    if bias is not None:
        bc = np.zeros((1, 128, rows), np.float32)
        bc[0, 0, :] = bias
        out = np.concatenate([out, bc], axis=0)
    return np.ascontiguousarray(out.astype(np.float16))


def _prep_inputs(x, W_ih, W_hh, b_ih, b_hh, W_out, b_out):
    in_maps = []
    e0 = np.zeros((128, 1), np.float32)
    e0[0, 0] = 1.0
    xdev = np.concatenate([x.reshape(128, KC), e0], axis=1).astype(np.float16)
    for c in range(N_CORES):
        sl = slice(c * HS, (c + 1) * HS)
        m = {}
        for l, (ih_name, hh_name) in enumerate((("wih0", "whh0"), ("wih1", "whh1"))):
            Wi = W_ih[l].reshape(4, H, H)[:, sl, :].reshape(NG, H)
            Wh = W_hh[l].reshape(4, H, H)[:, sl, :].reshape(NG, H)
            b = (b_ih[l] + b_hh[l]).reshape(4, H)[:, sl].reshape(NG)
            m[ih_name] = _pack_mat(Wi, b)
            m[hh_name] = _pack_mat(Wh)
        m["wout"] = _pack_mat(W_out[sl, :], b_out[sl])
        m["x0"] = xdev
        in_maps.append(m)
    return in_maps


def run(trace=False, **inputs):
    from concourse.bass_utils import run_bass_kernel_spmd

    if "nc" not in _CACHE:
        _CACHE["nc"] = _build()
    nc = _CACHE["nc"]
    inputs = {k: np.asarray(v, dtype=np.float32) for k, v in inputs.items()}
    in_maps = _prep_inputs(**inputs)
    res = run_bass_kernel_spmd(
        nc, in_maps, core_ids=list(range(N_CORES)), trace=trace,
    )
    return res


def kernel(**inputs):
    res = run(trace=False, **inputs)
    return np.asarray(res.results[0]["preds"], dtype=np.float32)
